# revision 26
# baseline (speedup 1.0000x reference)
"""Trainium2 Bass kernel for a decoder block (self-attn + cross-attn + FFN).

Sharding: data-parallel over 8 shards = (batch b in 0..3, seq-half h in 0..1).
Each core processes 512 query tokens of one batch element. Keys are kept in
GLOBAL token order; the causal mask is per-core input data, so the SPMD
program is identical on all cores.

K/V-projection dedup (vs the pure-DP baseline): each core K/V-projects only
its OWN 512 kv tokens (for SA these are exactly its own query rows, so the
xqT input doubles as the kv input and the full xkvT load is dropped; for CA
each core projects its contiguous half of the context). The halves are then
exchanged with the pair partner through a DRAM AllGather (replica groups
[[0,1],[2,3],[4,5],[6,7]]) and both halves are read back into k_sb/v_sb in
global order — AllGather output is ordered by rank within the pair, so the
readback APs are rank-independent. This halves the K/V projection matmuls
(-2.1 GMAC/core, about -65us of PE time) at the cost of two pairwise 2MB
collectives whose latency is hidden under Q-projection / the other layer's
K/V-own work.

On-chip layout convention:
  feature-major tile: [feature_part(128) x token_free]  (matmul inputs)
  token-major tile:   [token_part(128) x feature_free]  (softmax rows, LN, residual)

All matmuls run bf16 x bf16 -> fp32 PSUM. Residual/LN path stays fp32
(except the SA residual source, loaded bf16). Softmax denominators come free
from the attention O-matmul: the stationary operand is a 2-block AP
[V_head(64 cols) | ones(64 cols)], so PSUM rows 0-63 hold O_head and rows
64-127 the denominator replicated; one DVE reciprocal straight off PSUM + one
multiply normalize during evacuation.

Performance structure:
  - 2-bank PSUM pair tiles everywhere: each evacuation (exp / copy / relu /
    residual-add) is one wide ACT/DVE instruction.
  - Head pairs 2j/2j+1 live in K/Q partition halves 0-63/64-127, so their
    score matmuls alternate PE row groups (tile_position (0,0)/(64,0)) and
    overlap on the array.
  - Causal masking is multiplicative AFTER exp: one batched bf16 multiply
    per head over the 8 diagonal blocks via a 4-dim strided AP.
  - Transposes for the feature-major copy of x are emitted after ALL Oproj
    psum groups; 8 per 2-bank psum tile with a single wide evacuation.
  - DMA issue order follows consumption order; output is written bf16 and
    upcast on host.
"""

import os
import sys

for _p in ("/opt/trn_rl_repo",):
    if _p not in sys.path:
        sys.path.insert(0, _p)

import numpy as np
import ml_dtypes

import concourse.bass as bass
import concourse.tile as tile
from concourse import bacc, mybir
from concourse.ap import AP
from concourse.bass import ts
from concourse.masks import make_identity

E = 1024          # model dim
T = 512           # query tokens per core
TC = 1024         # kv tokens
H = 16            # heads
S = 64            # head dim
HID = 4096        # ffn hidden
EPS = 1e-5
SCALE2 = float(E) ** -0.5   # e^-0.25 applied to q AND k == e^-0.5 on scores

BF16 = mybir.dt.bfloat16
F32 = mybir.dt.float32

ET = E // 128     # 8 feature tiles
TT = T // 128     # 4 query-token tiles
CT = TC // 128    # 8 key-token tiles
NCH = E // 512    # 2 psum-width chunks of the feature dim
HT = HID // 128   # 32 hidden tiles

WNAMES = ["sa_wq", "sa_wk", "sa_wv", "sa_wo", "ca_wq", "ca_wk", "ca_wv", "ca_wo"]

REPLICA_PAIRS = [[0, 1], [2, 3], [4, 5], [6, 7]]

# cc staging layout (bf16, per partition): [ET, 512] own-token feature-major
# K = 4096 elem = 8KB/partition = 1MB per core. Only K is exchanged; V is
# recomputed in full on every core (cheaper than the collective's latency).
CCW = ET * 512


def _k_own_cc(nc, tc, name, kvin_own, wk_dram, cc_pieces, stage_pool,
              wpool, pools_pp, kvin_load=None):
    """Project K for this core's OWN 512 kv tokens (feature-major), stage to
    DRAM, and issue the pairwise AllGather(s). kvin_own(k) -> [128, 512]
    bf16 feature-major own-kv-input tile k. kvin_load(k), if given, emits
    the DMA for input tile k, interleaved with the wk tiles.

    cc_pieces: list of (cc_in, cc_out) DRAM AP pairs splitting the ET
    feature tiles evenly; each piece's AllGather is issued as soon as its
    share of K is staged, so the first head-pairs' keys arrive while the
    rest are still in flight."""
    pp = pools_pp
    npc = len(cc_pieces)
    mh = ET // npc  # feature tiles per piece
    wk_sb = wpool.tile([128, ET, E], BF16, tag="w", name=f"{name}_wk")
    for m in range(ET):
        if kvin_load is not None:
            kvin_load(m)
        nc.sync.dma_start(out=wk_sb[:, m, :], in_=wk_dram[ts(m, 128), :])
    for mp in range(0, ET, 2):
        ps2 = pp.tile([128, 2, 512], F32, tag="sc", name=f"{name}_psk")
        for c in range(2):
            for k in range(ET):
                nc.tensor.matmul(ps2[:, c, :],
                                 lhsT=wk_sb[:, k, ts(mp + c, 128)],
                                 rhs=kvin_own(k),
                                 start=(k == 0), stop=(k == ET - 1))
        stg = stage_pool.tile([128, 2, 512], BF16, tag="stage", bufs=2,
                              name=f"{name}_stgk{mp}")
        nc.scalar.copy(stg, ps2)
        pc = mp // mh
        off = (mp - pc * mh) * 512
        nc.sync.dma_start(out=cc_pieces[pc][0][:, off: off + 1024], in_=stg)
        if mp + 2 == (pc + 1) * mh:
            _cc_allgather(nc, cc_pieces[pc][0], cc_pieces[pc][1])


def _v_full(nc, tc, name, kvT_dram, wv_sb, v_sb, chunk_pool, pools_pp):
    """V projection over ALL kv tokens (both halves), token-major with the
    head-interleaved [V_h | ones] layout. The feature-major kv input is
    streamed from DRAM in [128, ET, 128] per-token-tile chunks (2KB/part)
    instead of holding the full 16KB xkvT resident."""
    pp = pools_pp
    for t in range(CT):
        chunk = chunk_pool.tile([128, ET, 128], BF16, tag="chk", bufs=2,
                                name=f"{name}_chk{t}")
        for k in range(ET):
            nc.sync.dma_start(out=chunk[:, k, :],
                              in_=kvT_dram[ts(k, 128), ts(t, 128)])
        ps2 = pp.tile([128, 2, 512], F32, tag="sc", name=f"{name}_psv")
        for k in range(ET):
            for c in range(NCH):
                mm = nc.tensor.matmul(ps2[:, c, :],
                                      lhsT=chunk[:, k, :],
                                      rhs=wv_sb[:, k, ts(c, 512)],
                                      start=(k == 0), stop=(k == ET - 1))
                if c > 0:
                    mm.ins.ldweights = False
        nc.scalar.copy(v_sb[:, t, :, 0:64],
                       ps2.rearrange("p c (j s) -> p (c j) s", j=8))


NO_CC = bool(int(os.environ.get("KERNEL_NO_CC", "0")))  # timing probe only


def _cc_allgather(nc, cc_in, cc_out):
    if NO_CC:
        return None
    return nc.gpsimd.collective_compute(
        "AllGather", mybir.AluOpType.bypass,
        replica_groups=REPLICA_PAIRS,
        ins=[cc_in[:]], outs=[cc_out[:]],
    )


def _k_readback(nc, name, cc_pieces, k_sb, interleaved):
    """Load both pair-halves of the exchanged K into k_sb (global token
    order). interleaved=True (SA): member ph owns blocks {ph, ph+2, ...} so
    the dest is a stride-256 view; False (CA): contiguous halves. Both are
    rank-independent: cc_out[ph] is member ph's data on every core."""
    npc = len(cc_pieces)
    mh = ET // npc
    for pc, (_, cc_out) in enumerate(cc_pieces):
        for ph in range(2):
            src = cc_out[ph]
            for mm_ in range(mh):
                m = pc * mh + mm_
                sr = src[:, mm_ * 512: (mm_ + 1) * 512].rearrange(
                    "p (b e) -> p b e", b=4)
                if interleaved:
                    base = k_sb[:, m, :]
                    d = AP(tensor=base.tensor, offset=base.offset + ph * 128,
                           ap=[list(base.ap[0]), [256, 4], [1, 128]])
                else:
                    d = k_sb[:, m, ph * 512: (ph + 1) * 512].rearrange(
                        "p (b e) -> p b e", b=4)
                nc.sync.dma_start(out=d, in_=sr)


def _attn_ln(nc, tc, name, qin, w_dram, mask_sb, resid_fn, xout_sb,
             xoutT_sb, id_f32, eps_sb, k_sb, v_sb, causal=False,
             preload=None, post_q=None, post_attn=None):
    """Attention (with k_sb/v_sb produced externally) + residual + layernorm.

    qin(k)  -> [128, T] bf16 feature-major query-input tile k
    w_dram  -> dict with wq, wo DRAM APs (natural [E, E] bf16)
    mask_sb -> [128, CT, 128] packed mask tile or None (causal only)
    resid_fn(t) -> [128, E] f32 token-major residual tile
    xout_sb -> [128, TT, E] f32 destination (post-LN, token-major)
    xoutT_sb-> [128, ET, T] bf16 destination (post-LN, feature-major) or None
    post_q  -> emitted after the Q projection (overlap work for the CC)
    post_attn-> emitted after the last O matmul (e.g. next readback/prefetch)
    """
    from contextlib import ExitStack

    with ExitStack() as st:
        wp = st.enter_context(tc.tile_pool(name=f"{name}_w", bufs=2))
        qp = st.enter_context(tc.tile_pool(name=f"{name}_q", bufs=1))
        ap_ = st.enter_context(tc.tile_pool(name=f"{name}_at", bufs=2))
        op = st.enter_context(tc.tile_pool(name=f"{name}_ot", bufs=1))
        xp = st.enter_context(tc.tile_pool(name=f"{name}_xr", bufs=2))
        sp = st.enter_context(tc.tile_pool(name=f"{name}_st", bufs=4))
        pp = st.enter_context(tc.tile_pool(name=f"{name}_ps", bufs=3, space="PSUM"))

        # ---- Q = (Xq @ Wq) * scale, feature-major [e_out, tq]
        wq_sb = wp.tile([128, ET, E], BF16, tag="w", name=f"{name}_wq")
        for m in range(ET):
            nc.sync.dma_start(out=wq_sb[:, m, :], in_=w_dram["wq"][ts(m, 128), :])
        q_sb = qp.tile([128, ET, T], BF16, name=f"{name}_qsb")
        for m in range(0, ET, 2):
            ps2 = pp.tile([128, 2, 512], F32, tag="sc", name=f"{name}_psq")
            for c in range(2):
                for k in range(ET):
                    nc.tensor.matmul(ps2[:, c, :],
                                     lhsT=wq_sb[:, k, ts(m + c, 128)],
                                     rhs=qin(k),
                                     start=(k == 0), stop=(k == ET - 1))
            nc.scalar.mul(q_sb[:, m: m + 2, :], ps2, SCALE2)

        if post_q is not None:
            post_q(pp, wp)
        if preload is not None:
            preload()

        # ---- per-head-pair: scores (transposed), exp, O with fused
        # denominator. Heads 2j / 2j+1 live in K/Q partition halves 0-63 /
        # 64-127; their score matmuls alternate PE row groups. Software-
        # pipelined: pair j's scores+exp are emitted before pair j-1's
        # O-matmuls.
        ot_sb = op.tile([128, ET, T], BF16, name=f"{name}_otsb")
        at_tiles = [None, None]

        def lo_of(i):
            return 128 * (i // 2) if causal else 0

        def apply_mask(at):
            atf = at[:, :, :]
            diag = AP(tensor=atf.tensor, offset=atf.offset,
                      ap=[list(atf.ap[0]), [2 * T + 128, CT // 2],
                          [T, 2], [1, 128]])
            nc.vector.tensor_mul(
                diag, diag,
                mask_sb[:].rearrange("p (a s) c -> p a s c", a=CT // 2))

        def scores2(j):
            atA = ap_.tile([128, CT, T], BF16, tag="at", bufs=4,
                           name=f"{name}_atA")
            atB = ap_.tile([128, CT, T], BF16, tag="at", bufs=4,
                           name=f"{name}_atB")
            at_tiles[j % 2] = (atA, atB)
            for p in range(CT // 2):
                i0 = 2 * p
                lo = 128 * p if causal else 0
                psA = pp.tile([128, 2, 512], F32, tag="sc", name=f"{name}_psA")
                psB = pp.tile([128, 2, 512], F32, tag="sc", name=f"{name}_psB")
                for s in range(2):
                    nc.tensor.matmul(psA[:, s, lo:512],
                                     lhsT=k_sb[0:64, j, ts(i0 + s, 128)],
                                     rhs=q_sb[0:64, j, lo:T],
                                     start=True, stop=True)
                    nc.tensor.matmul(psB[:, s, lo:512],
                                     lhsT=k_sb[64:128, j, ts(i0 + s, 128)],
                                     rhs=q_sb[64:128, j, lo:T],
                                     start=True, stop=True)
                nc.scalar.activation(atA[:, i0: i0 + 2, lo:T],
                                     psA[:, :, lo:512],
                                     func=mybir.ActivationFunctionType.Exp)
                nc.scalar.activation(atB[:, i0: i0 + 2, lo:T],
                                     psB[:, :, lo:512],
                                     func=mybir.ActivationFunctionType.Exp)
            if mask_sb is not None:
                apply_mask(atA)
                apply_mask(atB)

        def ovalue2(j):
            atA, atB = at_tiles[j % 2]
            for h, at in ((2 * j, atA), (2 * j + 1, atB)):
                pm, po = 64 * (h % 2), h // 2
                ps_o = pp.tile([128, T], F32, tag="oo", bufs=2,
                               name=f"{name}_pso")
                for i in range(CT):
                    lo = lo_of(i)
                    nc.tensor.matmul(ps_o[:, lo:T], lhsT=v_sb[:, i, h, :],
                                     rhs=at[:, i, lo:T],
                                     start=(i == 0), stop=(i == CT - 1))
                den = ap_.tile([64, T], F32, tag="den", name=f"{name}_den")
                nc.vector.reciprocal(den, ps_o[64:128, :])
                nc.vector.tensor_mul(ot_sb[pm: pm + 64, po, :],
                                     ps_o[0:64, :], den)

        # wo DMAs before the head loop: SP idle during the attention phase.
        wo_sb = wp.tile([128, ET, E], BF16, tag="w", name=f"{name}_wo")
        for m in range(ET):
            nc.sync.dma_start(out=wo_sb[:, m, :], in_=w_dram["wo"][ts(m, 128), :])

        scores2(0)
        for j in range(1, H // 2):
            scores2(j)
            ovalue2(j - 1)
        ovalue2(H // 2 - 1)

        if post_attn is not None:
            post_attn(pp, wp)
        for t in range(TT):
            xr = xp.tile([128, E], F32, tag="xr", name=f"{name}_xr")
            ps2 = pp.tile([128, 2, 512], F32, tag="sc", name=f"{name}_psw")
            for k in range(ET):
                for c in range(NCH):
                    mm = nc.tensor.matmul(ps2[:, c, :],
                                          lhsT=ot_sb[:, k, ts(t, 128)],
                                          rhs=wo_sb[:, k, ts(c, 512)],
                                          start=(k == 0), stop=(k == ET - 1))
                    if c > 0:
                        mm.ins.ldweights = False
            nc.vector.tensor_add(xr[:].rearrange("p (c s) -> p c s", c=2), ps2,
                                 resid_fn(t)[:, :].rearrange("p (c s) -> p c s",
                                                             c=2))
            _ln(nc, tc, name, t, xr, xout_sb, sp, eps_sb)
        if xoutT_sb is not None:
            for t in range(TT):
                # bf16 psum; padded to the "sc" ring slot size (4KB)
                pst = pp.tile([128, 2 * ET, 128], BF16, tag="sc", bufs=3,
                              name=f"{name}_ptr")
                for m in range(ET):
                    nc.tensor.transpose(pst[:, m, :],
                                        xout_sb[:, t, ts(m, 128)], id_f32)
                nc.scalar.copy(xoutT_sb[:, :, ts(t, 128)], pst[:, 0:ET, :])


def _ln(nc, tc, name, t, xr, xout_sb, sp, eps_sb):
    """LayerNorm of xr [128, E] f32 -> xout_sb[:, t, :]. gamma=1, beta=0."""
    stats = sp.tile([128, 2, 6], F32, tag="st", name=f"{name}_stats")
    for g in range(2):
        nc.vector.bn_stats(stats[:, g, :], xr[:, ts(g, 512)])
    mv = sp.tile([128, 2], F32, tag="mv", name=f"{name}_mv")
    nc.vector.bn_aggr(mv, stats)
    rstd = sp.tile([128, 1], F32, tag="rs", name=f"{name}_rstd")
    nc.scalar.activation(rstd, mv[:, 1:2],
                         func=mybir.ActivationFunctionType.Sqrt,
                         bias=eps_sb, scale=1.0)
    nc.vector.reciprocal(rstd, rstd)
    nc.vector.tensor_scalar(xout_sb[:, t, :], xr, mv[:, 0:1], rstd,
                            op0=mybir.AluOpType.subtract,
                            op1=mybir.AluOpType.mult)


def _emit(nc, tc, din, dout, cc, pfx=""):
    from contextlib import ExitStack

    with ExitStack() as top:
        const = top.enter_context(tc.tile_pool(name=f"{pfx}const", bufs=1))
        xtp = top.enter_context(tc.tile_pool(name=f"{pfx}xt", bufs=2))
        mp = top.enter_context(tc.tile_pool(name=f"{pfx}mask", bufs=1))
        rp = top.enter_context(tc.tile_pool(name=f"{pfx}resid", bufs=2))
        rtp = top.enter_context(tc.tile_pool(name=f"{pfx}residT", bufs=1))
        stp = top.enter_context(tc.tile_pool(name=f"{pfx}stage", bufs=1))
        kp = top.enter_context(tc.tile_pool(name=f"{pfx}ksb", bufs=1))
        chkp = top.enter_context(tc.tile_pool(name=f"{pfx}chk", bufs=2))

        id_f32 = const.tile([128, 128], BF16, name=f"{pfx}id_bf16")
        make_identity(nc, id_f32)
        eps_sb = const.tile([128, 1], F32, name=f"{pfx}eps_sb")
        nc.vector.memset(eps_sb, EPS)

        mask_sb = mp.tile([128, CT, 128], BF16, name=f"{pfx}mask_sb")
        x1_sb = rp.tile([128, TT, E], BF16, tag="x", name=f"{pfx}x1_sb")
        x1T_sb = rtp.tile([128, ET, T], BF16, tag="xT", name=f"{pfx}x1T_sb")
        attn_scope = top.enter_context(ExitStack())
        vp = attn_scope.enter_context(tc.tile_pool(name=f"{pfx}vsb", bufs=1))
        v_sb = vp.tile([128, CT, H, 128], BF16, name=f"{pfx}v_sb")
        nc.vector.memset(v_sb[:, :, :, 64:128], 1.0)

        # own-query (== own SA kv) input, feature-major; persists through
        # SA kv-own projection AND SA Q projection. DMAs are emitted by
        # _k_own_cc, interleaved with the wk tiles.
        xqT_sb = xtp.tile([128, ET, T], BF16, tag="xt", bufs=1,
                          name=f"{pfx}xqT_sb")

        def xqT_load(k):
            nc.sync.dma_start(out=xqT_sb[:, k, :], in_=din["xqT"][ts(k, 128), :])

        # xq residual tiles are SA-only; LIFO-scoped inside attn_scope
        sa_scope = ExitStack()
        xqp = sa_scope.enter_context(tc.tile_pool(name=f"{pfx}xq", bufs=2))
        xq_tiles = [xqp.tile([128, E], BF16, tag="xq", name=f"{pfx}xq_{t}")
                    for t in range(TT)]

        # ---- SA: K-own + exchange (keys stay in global token order; this
        # core owns the interleaved blocks {h, h+2, h+4, h+6}), then V over
        # ALL tokens locally. Scoped pools free their space before
        # _attn_ln's pools are created.
        with tc.tile_pool(name=f"{pfx}kvps", bufs=3, space="PSUM") as sa_pp, \
             tc.tile_pool(name=f"{pfx}kvw", bufs=2) as sa_kvwp:
            _k_own_cc(nc, tc, f"{pfx}sakv", lambda k: xqT_sb[:, k, :],
                      din["sa_wk"], cc["sa"], stp, sa_kvwp, sa_pp,
                      kvin_load=xqT_load)
            wv_sb = sa_kvwp.tile([128, ET, E], BF16, tag="w",
                                 name=f"{pfx}sa_wv")
            for m in range(ET):
                nc.sync.dma_start(out=wv_sb[:, m, :],
                                  in_=din["sa_wv"][ts(m, 128), :])
            _v_full(nc, tc, f"{pfx}sav", din["xkvT"], wv_sb, v_sb, chkp,
                    sa_pp)
            # CA K-own + its collective, still inside the scoped pools: the
            # CA collective enters the (serial) CC engine right behind the
            # SA one instead of mid-attention, so its result lands well
            # before the CA scores need it.
            ctxTh_sb = sa_kvwp.tile([128, ET, 512], BF16, tag="cth", bufs=1,
                                    name=f"{pfx}ctxTh_sb")

            def ctxTh_load(k):
                nc.sync.dma_start(out=ctxTh_sb[:, k, :],
                                  in_=din["ctxTh"][ts(k, 128), :])

            _k_own_cc(nc, tc, f"{pfx}cakv", lambda k: ctxTh_sb[:, k, :],
                      din["ca_wk"], cc["ca"], stp, sa_kvwp, sa_pp,
                      kvin_load=ctxTh_load)
        k_sb = kp.tile([128, ET, TC], BF16, name=f"{pfx}sa_ksb")
        _k_readback(nc, f"{pfx}sakv", cc["sa"], k_sb, interleaved=True)

        def sa_preload():
            for i in range(CT):
                nc.sync.dma_start(out=mask_sb[:, i, :],
                                  in_=din["maskT"][ts(i, 128), :])
            for t in range(TT):
                nc.sync.dma_start(out=xq_tiles[t], in_=din["xq"][ts(t, 128), :])

        ca_k_sb = [None]

        def sa_post_attn(attn_pp, attn_wp):
            # SA's last v_sb/k_sb readers are done: CA V over all tokens,
            # and pull in the exchanged CA K halves.
            wv_ca = attn_wp.tile([128, ET, E], BF16, tag="w",
                                 name=f"{pfx}ca_wv")
            for m in range(ET):
                nc.sync.dma_start(out=wv_ca[:, m, :],
                                  in_=din["ca_wv"][ts(m, 128), :])
            _v_full(nc, tc, f"{pfx}cav", din["ctxT"], wv_ca, v_sb, chkp,
                    attn_pp)
            ca_k_sb[0] = kp.tile([128, ET, TC], BF16, name=f"{pfx}ca_ksb")
            _k_readback(nc, f"{pfx}cakv", cc["ca"], ca_k_sb[0],
                        interleaved=False)

        _attn_ln(nc, tc, f"{pfx}sa",
                 qin=lambda k: xqT_sb[:, k, :],
                 w_dram={"wq": din["sa_wq"], "wo": din["sa_wo"]},
                 mask_sb=mask_sb,
                 resid_fn=lambda t: xq_tiles[t],
                 xout_sb=x1_sb, xoutT_sb=x1T_sb,
                 id_f32=id_f32, eps_sb=eps_sb,
                 k_sb=k_sb, v_sb=v_sb, causal=True,
                 preload=sa_preload, post_attn=sa_post_attn)
        sa_scope.close()  # frees the xq residual tiles (SA-only)

        x2_sb = rp.tile([128, TT, E], BF16, tag="x", name=f"{pfx}x2_sb")
        x2T_sb = rtp.tile([128, ET, T], BF16, tag="xT", name=f"{pfx}x2T_sb")

        _attn_ln(nc, tc, f"{pfx}ca",
                 qin=lambda k: x1T_sb[:, k, :],
                 w_dram={"wq": din["ca_wq"], "wo": din["ca_wo"]},
                 mask_sb=None,
                 resid_fn=lambda t: x1_sb[:, t, :],
                 xout_sb=x2_sb, xoutT_sb=x2T_sb,
                 id_f32=id_f32, eps_sb=eps_sb,
                 k_sb=ca_k_sb[0], v_sb=v_sb)
        attn_scope.close()

        # ---- FFN + residual + LN3 -> out
        with ExitStack() as st:
            wp = st.enter_context(tc.tile_pool(name=f"{pfx}ffw", bufs=1))
            hp = st.enter_context(tc.tile_pool(name=f"{pfx}ffh", bufs=1))
            xp = st.enter_context(tc.tile_pool(name=f"{pfx}ffxr", bufs=2))
            sp = st.enter_context(tc.tile_pool(name=f"{pfx}ffst", bufs=4))
            outp = st.enter_context(tc.tile_pool(name=f"{pfx}outp", bufs=2))
            pp = st.enter_context(tc.tile_pool(name=f"{pfx}ffps", bufs=4,
                                               space="PSUM"))

            HH = HT // 2  # 16 hidden tiles per half
            ffh_sb = hp.tile([128, HT, T], BF16, name=f"{pfx}ffh_sb")
            w2_halves = []
            for p_ in range(2):
                w1h = wp.tile([128, ET, HH * 128], BF16, tag="fw", bufs=2,
                              name=f"{pfx}w1_sb{p_}")
                for m in range(ET):
                    nc.sync.dma_start(
                        out=w1h[:, m, :],
                        in_=din["ff_w1"][ts(m, 128), ts(p_, HH * 128)])
                for mm_ in range(0, HH, 2):
                    m = p_ * HH + mm_
                    ps2 = pp.tile([128, 2, 512], F32, tag="sc", name=f"{pfx}ffps1")
                    for c in range(2):
                        for k in range(ET):
                            nc.tensor.matmul(ps2[:, c, :],
                                             lhsT=w1h[:, k, ts(mm_ + c, 128)],
                                             rhs=x2T_sb[:, k, :],
                                             start=(k == 0), stop=(k == ET - 1))
                    nc.scalar.activation(ffh_sb[:, m: m + 2, :], ps2,
                                         func=mybir.ActivationFunctionType.Relu)
            for p_ in range(2):
                w2h = wp.tile([128, HH, E], BF16, tag="fw", bufs=2,
                              name=f"{pfx}w2_sb{p_}")
                for mm_ in range(HH):
                    nc.sync.dma_start(out=w2h[:, mm_, :],
                                      in_=din["ff_w2"][ts(p_ * HH + mm_, 128), :])
                w2_halves.append(w2h)
            for t in range(TT):
                xr = xp.tile([128, E], F32, tag="xr", name=f"{pfx}ff_xr")
                ps2 = pp.tile([128, 2, 512], F32, tag="sc", name=f"{pfx}ffps2")
                for m in range(HT):
                    for c in range(NCH):
                        mm = nc.tensor.matmul(
                            ps2[:, c, :],
                            lhsT=ffh_sb[:, m, ts(t, 128)],
                            rhs=w2_halves[m // HH][:, m % HH, ts(c, 512)],
                            start=(m == 0), stop=(m == HT - 1))
                        if c > 0:
                            mm.ins.ldweights = False
                nc.vector.tensor_add(xr[:].rearrange("p (c s) -> p c s", c=2),
                                     ps2,
                                     x2_sb[:, t, :].rearrange("p (c s) -> p c s",
                                                              c=2))
                out_t = outp.tile([128, E], BF16, tag="out", name=f"{pfx}out_t")
                _ln(nc, tc, f"{pfx}ff", 0, xr,
                    out_t.rearrange("p (o e) -> p o e", o=1), sp, eps_sb)
                nc.sync.dma_start(out=dout[ts(t, 128), :], in_=out_t)


def build_program(n_iters=1):
    """n_iters>1 python-unrolls the body (collectives are not supported
    inside hardware For loops); used only for benchmarking."""
    nc = bacc.Bacc(num_devices=8)
    din = {}

    def inp(name, shape, dt):
        din[name] = nc.dram_tensor(name, shape, dt, kind="ExternalInput").ap()

    inp("xq", [T, E], BF16)
    inp("xqT", [E, T], BF16)
    inp("xkvT", [E, TC], BF16)
    inp("ctxT", [E, TC], BF16)
    inp("ctxTh", [E, 512], BF16)
    inp("maskT", [TC, 128], BF16)
    for w in WNAMES:
        inp(w, [E, E], BF16)
    inp("ff_w1", [E, HID], BF16)
    inp("ff_w2", [HID, E], BF16)
    dout = nc.dram_tensor("out", [T, E], BF16, kind="ExternalOutput").ap()

    def mk_cc(i):
        sfx = f"_{i}" if i else ""

        def piece(nm, w):
            return (nc.dram_tensor(f"{nm}_in{sfx}", [128, w], BF16,
                                   kind="Internal").ap(),
                    nc.dram_tensor(f"{nm}_out{sfx}", [2, 128, w], BF16,
                                   kind="Internal").ap())

        return {
            "sa": [piece(f"sa_cc_a", CCW // 2), piece(f"sa_cc_b", CCW // 2)],
            "ca": [piece(f"ca_cc", CCW)],
        }

    with tile.TileContext(nc) as tc:
        if n_iters == 1:
            _emit(nc, tc, din, dout, mk_cc(0))
        else:
            for i in range(n_iters):
                _emit(nc, tc, din, dout, mk_cc(i), pfx=f"i{i}_")
    nc.compile()
    return nc


def own_rows(h):
    """Global token rows owned by seq-half h: interleaved 128-blocks
    {h, h+2, h+4, h+6} so the causal wavefront is balanced and key tile i
    is only needed by local query tiles j >= i//2."""
    return np.concatenate(
        [np.arange(128 * (2 * j + h), 128 * (2 * j + h) + 128) for j in range(TT)])


def shard_inputs(inputs):
    """Full inputs -> list of 8 per-core input maps."""
    bf = ml_dtypes.bfloat16
    x = np.asarray(inputs["x"], np.float32)
    ctx = np.asarray(inputs["context"], np.float32)
    wcast = {w: np.ascontiguousarray(np.asarray(inputs[w], np.float32).astype(bf))
             for w in WNAMES + ["ff_w1", "ff_w2"]}
    maps = []
    for c in range(8):
        b, h = divmod(c, 2)
        rows = own_rows(h)
        own = x[b, rows]                      # (T, E) own queries, token-major
        maskP = np.zeros((TC, 128), np.float32)
        for i in range(CT):
            g = 2 * (i // 2) + h
            kpos = 128 * i + np.arange(128)
            qpos = 128 * g + np.arange(128)
            maskP[128 * i: 128 * i + 128, :] = np.where(
                kpos[:, None] <= qpos[None, :], 1.0, 0.0)
        m = {
            "xq": np.ascontiguousarray(own.astype(bf)),
            "xqT": np.ascontiguousarray(own.T.astype(bf)),
            "xkvT": np.ascontiguousarray(x[b].T.astype(bf)),
            "ctxT": np.ascontiguousarray(ctx[b].T.astype(bf)),
            "ctxTh": np.ascontiguousarray(ctx[b, 512 * h: 512 * (h + 1)].T
                                          .astype(bf)),
            "maskT": np.ascontiguousarray(maskP.astype(bf)),
        }
        m.update(wcast)
        maps.append(m)
    return maps


def gather_outputs(results):
    out = np.empty((4, 1024, E), np.float32)
    for c in range(8):
        b, h = divmod(c, 2)
        out[b, own_rows(h)] = np.asarray(results[c]["out"], np.float32)
    return out


def kernel(**inputs):
    from concourse.bass_utils import run_bass_kernel_spmd

    nc = build_program()
    in_maps = shard_inputs(inputs)
    core_ids = list(range(8))
    res = run_bass_kernel_spmd(nc, in_maps, core_ids)
    return gather_outputs(res.results)


if __name__ == "__main__":
    nc = build_program()
    print("program built ok")
